# revision 24
# baseline (speedup 1.0000x reference)
"""Trainium2 Bass kernel for nn_DARTSModelLayers (FISTA-style unrolled model).

Math (per reference):
  W = frozen_weight[0]  [N=512, H=1024];  L = ||W||_2^2
  10 iterations of:
    z_aux = z + (i/(i+3)) (z - z_prev)
    z_g   = (I - W^T W / L) z_aux + W^T x / L
    z_op  = sum_k softmax(alpha_i)_k * op_k(z_g)        (20 activations)
    z_prev = bw0 z + bw1 z_op ; z = z_op

Per-iteration decomposition (u = z_g; psum = A@tmp + c/coef; u = coef*psum):
  ACT (6 passes, 2 table loads): sm=sigmoid(-u), e2=erf(u/sqrt2), tt=tanh(u)
    [sigmoid set]; lnsm=ln(sm), eu=exp(u+ln wE) [nat_log_exp set];
    th=tanh(-lnsm) [sigmoid set again -> next iter needs no load]
  DVE (TS 4x / TT 2x fp16): xh=coef*psum, prescales (1+|u|, clip, hardsig,
    (u+w17/w7), (-w18*u-w16)), products (uhw, tsm, ue2, uth), accd, evac.
  GPSIMD (SBUF-only STT chain): tmp, q1=cA*Aq1, q2+=clip term, q3+=c_tt*tt,
    q4+=c_ln*lnsm, q5+=min(eu,wE)  (exp(min(u,0))=min(exp(u),1) trick).
  PE: zg matmuls (A fp16 with I folded; c via scaled-identity matmul) and
    z_op term accumulation via scaled-identity matmuls into psum.

Softsign is approximated as GT*tanh(u) + GC*clip(u,-1,1), folded into the
tanh / hardtanh coefficients (golden.py: end-to-end rel err 4.3e-3 fp16).

Batch columns are independent through the whole recurrence, so each core
processes its 512 batch columns as CH=2 chunks of 256, each chunk owning
half of PSUM (4 banks). Chunk B's tail overlaps chunk A's next-iteration
matmuls, keeping all four engines busy across the serial iteration chain.

Sharding: batch B=4096 split over 8 cores (512 each); weights replicated.
Output is produced in [H, B_shard] fp16 layout; host transposes to [B,H,1].
"""
import sys
import numpy as np

sys.path.insert(0, "/opt/trn_rl_repo")

import concourse.bass as bass  # noqa: E402
import concourse.bacc as bacc  # noqa: E402
import concourse.tile as tile  # noqa: E402
from concourse import mybir  # noqa: E402
from concourse.bass_utils import run_bass_kernel_spmd  # noqa: E402
from contextlib import ExitStack  # noqa: E402

F32 = mybir.dt.float32
F16 = mybir.dt.float16
ACT = mybir.ActivationFunctionType
ALU = mybir.AluOpType

B, N, H, T = 4096, 512, 1024, 10
NCORES = 8
BS = B // NCORES          # 512 batch per core
NG = H // 128             # 8 h-tile groups
NJ = N // 128             # 4 n-tile blocks
FS = NG * BS              # 4096
CH = 2                    # batch chunks per core
GB = BS // CH             # 256 batch per chunk
CHF = NG * GB             # 2048 free per chunk tile
INV_SQRT2 = 0.7071067811865476
LAM_SELU = 1.0507009873554805
ALPHA_SELU = 1.6732632423543772
# softsign ~ GT*tanh(u) + GC*clip(u,-1,1)  (fit on [-3.2,3.2]; golden.py)
GT, GC = 0.9896, -0.2706


def _softmax(v):
    v = v - v.max()
    e = np.exp(v)
    return e / e.sum()


def _iter_consts(aw, bw):
    out = []
    for i in range(T):
        w = aw[i]
        c_r = w[1] + w[4] + w[9] + LAM_SELU * w[8] + 0.99 * w[10]
        wE = w[4] + w[9] + LAM_SELU * ALPHA_SELU * w[8]
        c_lin = (w[0] + w[2] + w[5] + 0.01 * w[10] + w[11] + w[12]
                 + 0.5 * w[3] + w[18])
        c_tt = w[15] - w[12] + GT * w[13]
        w6p = w[6] + GC * w[13]
        d = dict(
            c_lin=c_lin, c_r=c_r, wE=wE, ln_wE=float(np.log(wE)),
            c_tt=c_tt, c_ln=w[11] - w[14], w6p=w6p, aw6p=abs(w6p),
            w7=w[7], w17=w[17], w18=w[18], w16=w[16],
            hw3=0.5 * w[3], w19=w[19],
            Ktot=w[16] - wE,
        )
        if i == 0:
            d["coef"], d["szold"] = 1.0, 0.0
        else:
            mom = i / (i + 3.0)
            bwp = bw[i - 1]
            d["coef"] = 1.0 + mom * (1.0 - bwp[1])
            d["szold"] = (-mom * bwp[0]) / d["coef"]
        d["s_xh"] = d["c_lin"] - 1.0 / d["coef"]
        out.append(d)
    return out


def _build(L, aw, bw):
    nc = bacc.Bacc("TRN2", target_bir_lowering=False, debug=False,
                   num_devices=NCORES)
    cons = _iter_consts(aw, bw)

    w_lhs_d = nc.dram_tensor("w_lhs", [128, NJ * H], F16, kind="ExternalInput")
    w_rhs_d = nc.dram_tensor("w_rhs", [128, NJ * H], F16, kind="ExternalInput")
    xT_d = nc.dram_tensor("xT", [128, NJ * BS], F16, kind="ExternalInput")
    identp_d = nc.dram_tensor("identp", [128, 128], F16, kind="ExternalInput")
    identc_d = nc.dram_tensor("identc", [128, (T - 1) * 128], F16,
                              kind="ExternalInput")
    identa_d = nc.dram_tensor("identa", [128, 4 * T * 128], F16,
                              kind="ExternalInput")
    z_d = nc.dram_tensor("z_out", [H, BS], F16, kind="ExternalOutput")

    with tile.TileContext(nc) as tc, ExitStack() as ctx:
        ctx.enter_context(nc.allow_low_precision(
            reason="fp16 basis chain; validated vs jax reference in golden.py"))
        state = ctx.enter_context(tc.tile_pool(name="state", bufs=1))
        psp = ctx.enter_context(tc.tile_pool(name="ps", bufs=1, space="PSUM"))
        psc = [psp.tile([128, CHF], F32, name=f"psc{ch}") for ch in range(CH)]

        m_sb = state.tile([128, NG * H], F16, name="m_sb")
        identp = state.tile([128, 128], F16, name="identp")
        identc = state.tile([128, (T - 1) * 128], F16, name="identc")
        identa = state.tile([128, 4 * T * 128], F16, name="identa")
        c16 = [state.tile([128, CHF], F16, name=f"c16_{ch}") for ch in range(CH)]
        zP = [[state.tile([128, CHF], F16, name=f"z{k}_{ch}") for k in range(2)]
              for ch in range(CH)]
        tmp = [state.tile([128, CHF], F16, name=f"tmp_{ch}") for ch in range(CH)]
        ebias = state.tile([128, T], F32, name="ebias")
        for i in range(T):
            nc.vector.memset(ebias[:, i:i + 1], cons[i]["ln_wE"])
        nc.sync.dma_start(identp[:], identp_d[:, :])
        nc.sync.dma_start(identc[:], identc_d[:, :])
        nc.sync.dma_start(identa[:], identa_d[:, :])

        # ---------------- setup: M = I - W^T W/L (m_sb), c = W^T x/L -------
        with tc.tile_pool(name="setup", bufs=1) as sp:
            w_lhs = sp.tile([128, NJ * H], F16, name="w_lhs")
            w_rhs = sp.tile([128, NJ * H], F16, name="w_rhs")
            xT = sp.tile([128, NJ * BS], F16, name="xT")
            nc.sync.dma_start(w_lhs[:], w_lhs_d[:, :])
            nc.sync.dma_start(w_rhs[:], w_rhs_d[:, :])
            nc.sync.dma_start(xT[:], xT_d[:, :])

            # M in 2 waves of 8 chunks (g,half); wave wv -> m_sb[:, wv*4096:]
            # Uses both psum tiles as one 8-bank scratch.
            for wv in range(2):
                for cg in range(4):
                    g = wv * 4 + cg
                    for half in range(2):
                        bnk = cg * 2 + half
                        pt = psc[bnk // 4]
                        off = (bnk % 4) * 512
                        sl = pt[:, off: off + 512]
                        has_diag = (g * 128) // 512 == half
                        for j in range(NJ):
                            nc.tensor.matmul(
                                sl,
                                w_lhs[:, j * H + g * 128: j * H + g * 128 + 128],
                                w_rhs[:, j * H + half * 512: j * H + half * 512 + 512],
                                start=(j == 0), stop=(j == NJ - 1))
                            if j == 0 and has_diag:
                                doff = off + g * 128 - half * 512
                                nc.tensor.matmul(
                                    pt[:, doff: doff + 128], identp[:],
                                    identp[:], start=False, stop=False,
                                    skip_group_check=True)
                nc.scalar.copy(m_sb[:, wv * 4096: wv * 4096 + 2048], psc[0][:])
                nc.scalar.copy(m_sb[:, wv * 4096 + 2048: (wv + 1) * 4096],
                               psc[1][:])

            # c chunked: psc[ch] group g at [g*GB, (g+1)*GB)
            for ch in range(CH):
                for g in range(NG):
                    sl = psc[ch][:, g * GB:(g + 1) * GB]
                    for j in range(NJ):
                        nc.tensor.matmul(
                            sl,
                            w_lhs[:, j * H + g * 128: j * H + g * 128 + 128],
                            xT[:, j * BS + ch * GB: j * BS + ch * GB + GB],
                            start=(j == 0 and g % 2 == 0), stop=False,
                            skip_group_check=True)
                nc.scalar.copy(c16[ch][:], psc[ch][:])

        # ---------------- iterations ----------------
        work = ctx.enter_context(tc.tile_pool(name="work", bufs=1))

        def wt(tag, ch):
            return work.tile([128, CHF], F16, tag=f"{tag}_{ch}",
                             name=f"{tag}_{ch}")

        z_im1 = [None, None]
        z_im2 = [None, None]
        for i in range(T):
            cc = cons[i]
            coef = cc["coef"]
            tiles = [dict() for _ in range(CH)]

            # --- zg matmuls (PE); tmp[ch] was produced at the end of the
            # previous iteration directly from psum, so zg does not wait on
            # the previous evac.
            for ch in range(CH):
                if i == 0:
                    continue
                # start=True only on the FIRST matmul touching each psum
                # bank (even g): start clears has_written bank-wide, and the
                # odd sibling's fresh region then gets correct
                # overwrite-then-accumulate semantics with start=False.
                for g in range(NG):
                    sl = psc[ch][:, g * GB:(g + 1) * GB]
                    for j in range(NG):
                        nc.tensor.matmul(
                            sl,
                            m_sb[:, j * H + g * 128: j * H + g * 128 + 128],
                            tmp[ch][:, j * GB:(j + 1) * GB],
                            start=(j == 0 and g % 2 == 0), stop=False,
                            skip_group_check=True)
                    nc.tensor.matmul(
                        sl, identc[:, (i - 1) * 128: i * 128],
                        c16[ch][:, g * GB:(g + 1) * GB],
                        start=False, stop=False, skip_group_check=True)

            # --- DVE: xh + prescales (TS 4x), chunk-ordered ---
            for ch in range(CH):
                t = tiles[ch]
                t["xh"] = wt("xh", ch)
                nc.vector.tensor_scalar(t["xh"][:], psc[ch][:], coef, None,
                                        ALU.mult)
                t["relw"] = wt("relw", ch)
                nc.vector.tensor_scalar(t["relw"][:], t["xh"][:],
                                        cc["c_r"], 0.0, ALU.mult, ALU.max)
                t["c1a"] = wt("c1a", ch)
                nc.vector.tensor_scalar(t["c1a"][:], t["xh"][:], cc["w6p"],
                                        cc["aw6p"], ALU.mult, ALU.min)
                t["c1w"] = wt("c1w", ch)
                nc.vector.tensor_scalar(t["c1w"][:], t["c1a"][:],
                                        -cc["aw6p"], None, ALU.max)
                t["hm1"] = wt("uhw", ch)   # shares buffer w/ uhw (disjoint)
                nc.vector.tensor_scalar(t["hm1"][:], t["xh"][:],
                                        cc["w7"] / 6.0, cc["w7"] / 2.0,
                                        ALU.mult, ALU.add)
                t["hmw"] = wt("hmw", ch)
                nc.vector.tensor_scalar(t["hmw"][:], t["hm1"][:], cc["w7"],
                                        0.0, ALU.min, ALU.max)
                t["thsw"] = wt("thsw", ch)
                nc.vector.tensor_scalar(t["thsw"][:], t["xh"][:],
                                        cc["w17"] / cc["w7"], None, ALU.add)
                t["tsmt"] = wt("tsmt", ch)
                nc.vector.tensor_scalar(t["tsmt"][:], t["xh"][:], -cc["w18"],
                                        -cc["w16"], ALU.mult, ALU.add)
                t["xh35"] = wt("xh35", ch)
                nc.vector.tensor_scalar(t["xh35"][:], t["xh"][:], cc["hw3"],
                                        None, ALU.mult)

            # --- per chunk: ACT [S] -> [L] -> [S] phases, with DVE/GP
            # products interleaved.  Emission order keeps each chunk's set
            # phases contiguous; across chunks the trailing [S] (th) merges
            # with the next chunk's leading [S] block.
            for ch in range(CH):
                t = tiles[ch]
                # order: [2: sm, e2][6: eu, ln][2: tt, th] -- every
                # consumer is >=2 queue slots after its producer, so
                # semaphore-post latency never lets the scheduler shear an
                # op from another set into the middle of a phase.
                t["sm"] = wt("sm", ch)
                nc.scalar.activation(t["sm"][:], psc[ch][:], ACT.Sigmoid,
                                     scale=-coef)
                t["e2"] = wt("e2", ch)
                nc.scalar.activation(t["e2"][:], psc[ch][:], ACT.Erf,
                                     scale=coef * INV_SQRT2)
                t["eu"] = wt("eu", ch)
                nc.scalar.activation(t["eu"][:], psc[ch][:], ACT.Exp,
                                     scale=coef, bias=ebias[:, i:i + 1])
                t["lnsm"] = wt("lnsm", ch)
                nc.scalar.activation(t["lnsm"][:], t["sm"][:], ACT.Ln)
                t["tt"] = wt("tt", ch)
                nc.scalar.activation(t["tt"][:], psc[ch][:], ACT.Tanh,
                                     scale=coef)
                t["th"] = wt("th", ch)
                nc.scalar.activation(t["th"][:], t["lnsm"][:], ACT.Tanh,
                                     scale=-1.0)
                # DVE products + helpers for this chunk
                t["uhw"] = wt("uhw", ch)          # hm1 dead
                nc.vector.tensor_mul(t["uhw"][:], t["thsw"][:], t["hmw"][:])
                t["eum"] = wt("hmw", ch)          # hmw dead after uhw
                nc.vector.tensor_scalar(t["eum"][:], t["eu"][:], cc["wE"],
                                        None, ALU.min)
                t["uth"] = wt("uth", ch)
                nc.vector.tensor_mul(t["uth"][:], t["xh"][:], t["th"][:])
                t["ecr"] = wt("c1a", ch)          # c1a dead after c1w
                nc.vector.tensor_add(t["ecr"][:], t["c1w"][:], t["relw"][:])
                t["ecm"] = wt("relw", ch)         # relw dead after ecr
                nc.vector.tensor_add(t["ecm"][:], t["ecr"][:], t["eum"][:])
                if 0 < i < T - 1:
                    t["sz2"] = wt("th", ch)       # th dead after uth
                    nc.vector.tensor_scalar(
                        t["sz2"][:], z_im1[ch][:],
                        cons[i + 1]["szold"], None, ALU.mult)
                # GPSIMD products
                t["tsm"] = wt("tsm", ch)
                nc.gpsimd.tensor_mul(t["tsm"][:], t["tsmt"][:], t["sm"][:])
                t["ue2w"] = wt("ue2w", ch)
                nc.gpsimd.tensor_mul(t["ue2w"][:], t["xh35"][:], t["e2"][:])

            # --- PE z_op terms continue the open zg psum group (start=False
            # everywhere -> no bank-wide has_written clear -> k-outer order is
            # safe and reuses each stationary across all 8 groups).  The xh
            # stationary is (c_lin - 1/coef) so psum's zg content folds into
            # the linear term: psum + (c_lin-1/coef)*coef*psum = c_lin*u.
            ia = identa[:, (i * 4 + 0) * 128:(i * 4 + 1) * 128]   # c_lin-1/coef
            ib = identa[:, (i * 4 + 1) * 128:(i * 4 + 2) * 128]   # c_tt
            ic = identa[:, (i * 4 + 2) * 128:(i * 4 + 3) * 128]   # c_ln
            id_ = identa[:, (i * 4 + 3) * 128:(i * 4 + 4) * 128]  # w19
            for ch in range(CH):
                t = tiles[ch]
                terms = [(ia, t["xh"]), (ib, t["tt"]), (ic, t["lnsm"]),
                         (identp, t["ecm"]), (identp, t["uhw"]),
                         (identp, t["tsm"]), (identp, t["ue2w"]),
                         (id_, t["uth"])]
                for k, (ident_t, src_t) in enumerate(terms):
                    for g in range(NG):
                        nc.tensor.matmul(
                            psc[ch][:, g * GB:(g + 1) * GB],
                            ident_t[:], src_t[:, g * GB:(g + 1) * GB],
                            start=False, stop=(k == len(terms) - 1),
                            skip_group_check=True)

            # --- next-iter tmp straight from psum (parallel with evac) ---
            for ch in range(CH):
                if i == T - 1:
                    continue
                if i == 0:
                    nc.vector.tensor_scalar(tmp[ch][:], psc[ch][:], 1.0,
                                            cc["Ktot"], ALU.mult, ALU.add)
                else:
                    nc.vector.scalar_tensor_tensor(
                        tmp[ch][:], psc[ch][:], cc["Ktot"],
                        tiles[ch]["sz2"][:], ALU.add, ALU.add)

            # --- evac (ACT Copy; copy is in every table set) ---
            for ch in range(CH):
                z_out_t = zP[ch][i % 2]
                nc.scalar.activation(z_out_t[:], psc[ch][:], ACT.Copy,
                                     bias=cc["Ktot"])
                z_im2[ch] = z_im1[ch]
                z_im1[ch] = z_out_t

        for ch in range(CH):
            for g in range(NG):
                nc.sync.dma_start(
                    z_d[g * 128:(g + 1) * 128, ch * GB:(ch + 1) * GB],
                    z_im1[ch][:, g * GB:(g + 1) * GB])

    # Restrict the ACT table-set chooser to sigmoid_and_others (sigmoid,
    # erf, tanh, copy) and natural_log_exp_and_others (ln, exp): the default
    # first-containing-set policy maps tanh/ln to other sets, causing ~8
    # table loads per iteration instead of 2.  Indices are preserved (other
    # entries become empty), so walrus sees valid set ids.  The original
    # function is restored immediately after finalize.
    _orig_gat = bacc.get_activation_tables
    _keep = {"sigmoid_and_others", "natural_log_exp_and_others"}

    def _patched_gat(arch):
        return {name: (fns if name in _keep else set())
                for name, fns in _orig_gat(arch).items()}

    bacc.get_activation_tables = _patched_gat
    try:
        nc.finalize()
    finally:
        bacc.get_activation_tables = _orig_gat
    return nc


_CACHE = {}


def kernel(x, frozen_weight, alpha, layer_beta, _want_trace=False):
    x = np.asarray(x, np.float32)
    frozen_weight = np.asarray(frozen_weight, np.float32)
    alpha = np.asarray(alpha, np.float32)
    layer_beta = np.asarray(layer_beta, np.float32)

    W = frozen_weight[0]                                   # [N, H]
    L = float(np.linalg.norm(W.astype(np.float64), 2) ** 2)
    aw = np.stack([_softmax(alpha[i].astype(np.float64)) for i in range(T)])
    bw = np.stack([_softmax(layer_beta[i].astype(np.float64))
                   for i in range(T)])

    key = (round(L, 10), aw.tobytes(), bw.tobytes())
    if key not in _CACHE:
        _CACHE[key] = _build(L, aw, bw)
    nc = _CACHE[key]
    cons = _iter_consts(aw, bw)

    ident = np.eye(128, dtype=np.float16)
    identc = np.concatenate(
        [ident * np.float16(1.0 / cons[i]["coef"]) for i in range(1, T)],
        axis=1)
    identa = np.concatenate(
        [ident * np.float16(cons[i][k])
         for i in range(T) for k in ("s_xh", "c_tt", "c_ln", "w19")],
        axis=1)

    # w in n-block-partition layout: w_blk[p, j*H + h] = W[j*128+p, h]
    w_blk = W.reshape(NJ, 128, H).transpose(1, 0, 2).reshape(128, NJ * H)
    w_lhs = w_blk.astype(np.float16)
    w_rhs = (-w_blk / L).astype(np.float16)
    xs = x[:, :, 0]                                        # [B, N]

    in_maps = []
    for c in range(NCORES):
        xc = xs[c * BS:(c + 1) * BS, :]                    # [BS, N]
        xT = (xc.T / L).reshape(NJ, 128, BS).transpose(1, 0, 2) \
            .reshape(128, NJ * BS).astype(np.float16)
        in_maps.append({
            "w_lhs": np.ascontiguousarray(w_lhs),
            "w_rhs": np.ascontiguousarray(w_rhs),
            "xT": np.ascontiguousarray(xT),
            "identp": ident,
            "identc": np.ascontiguousarray(identc),
            "identa": np.ascontiguousarray(identa),
        })

    res = run_bass_kernel_spmd(nc, in_maps, list(range(NCORES)),
                               trace=_want_trace)
    # z_out [H, BS] but batch columns are chunk-grouped: column layout is
    # [ch*GB + b] = batch (ch*GB + b)  -- contiguous, nothing to undo.
    z = np.concatenate([np.asarray(res.results[c]["z_out"], np.float32)
                        for c in range(NCORES)], axis=1)
    out = np.ascontiguousarray(z.T)[:, :, None].astype(np.float32)
    if _want_trace:
        return out, res
    return out


if __name__ == "__main__":
    d = np.load('/tmp/inputs.npz')
    out = kernel(d['x'], d['frozen_weight'], d['alpha'], d['layer_beta'])
    ref = np.load('/tmp/ref_out.npy')
    rel = np.linalg.norm(out - ref) / np.linalg.norm(ref)
    print("rel err vs ref:", rel, "absmax:", np.abs(out - ref).max())


# revision 26
# speedup vs baseline: 1.0544x; 1.0544x over previous
"""Trainium2 Bass kernel for nn_DARTSModelLayers (FISTA-style unrolled model).

Math (per reference):
  W = frozen_weight[0]  [N=512, H=1024];  L = ||W||_2^2
  10 iterations of:
    z_aux = z + (i/(i+3)) (z - z_prev)
    z_g   = (I - W^T W / L) z_aux + W^T x / L
    z_op  = sum_k softmax(alpha_i)_k * op_k(z_g)        (20 activations)
    z_prev = bw0 z + bw1 z_op ; z = z_op

Per-iteration decomposition (u = z_g; psum = A@tmp + c/coef; u = coef*psum):
  ACT (6 passes, 2 table loads): sm=sigmoid(-u), e2=erf(u/sqrt2), tt=tanh(u)
    [sigmoid set]; lnsm=ln(sm), eu=exp(u+ln wE) [nat_log_exp set];
    th=tanh(-lnsm) [sigmoid set again -> next iter needs no load]
  DVE (TS 4x / TT 2x fp16): xh=coef*psum, tmp (STT), prescales (c_r*relu,
    clip, hardsig, (u+w17/w7), (-w18*u-w16), hw3*u), eum=min(eu,wE) (the
    exp(min(u,0))=min(exp(u),1) trick), products uhw/uth, ecr=c1w+relw.
  GPSIMD (TENSOR_TENSOR only on TRN2): products tsm, ue2w.
  PE: zg matmuls (A fp16 with I folded; c via scaled-identity matmul), then
    z_op terms continue the same psum accumulation group via scaled-identity
    matmuls (start=True only on each bank's first touch; the xh stationary
    is c_lin-1/coef so the zg psum content folds into the linear term).
  ACT order [sm,e2][eu,ln][tt,th] keeps every consumer >=2 slots behind its
  producer so the scheduler cannot shear table-set phases (4 loads/iter);
  the set chooser is restricted to sigmoid_and_others/natural_log_exp via a
  scoped get_activation_tables patch around finalize.  Evac is an ACT Copy
  with bias=Ktot.

Softsign is approximated as GT*tanh(u) + GC*clip(u,-1,1), folded into the
tanh / hardtanh coefficients (golden.py: end-to-end rel err 4.3e-3 fp16).

Batch columns are independent through the whole recurrence, so each core
processes its 512 batch columns as CH=2 chunks of 256, each chunk owning
half of PSUM (4 banks). Chunk B's tail overlaps chunk A's next-iteration
matmuls, keeping all four engines busy across the serial iteration chain.

Sharding: batch B=4096 split over 8 cores (512 each); weights replicated.
Output is produced in [H, B_shard] fp16 layout; host transposes to [B,H,1].
"""
import sys
import numpy as np

sys.path.insert(0, "/opt/trn_rl_repo")

import concourse.bass as bass  # noqa: E402
import concourse.bacc as bacc  # noqa: E402
import concourse.tile as tile  # noqa: E402
from concourse import mybir  # noqa: E402
from concourse.bass_utils import run_bass_kernel_spmd  # noqa: E402
from contextlib import ExitStack  # noqa: E402

F32 = mybir.dt.float32
F16 = mybir.dt.float16
ACT = mybir.ActivationFunctionType
ALU = mybir.AluOpType

B, N, H, T = 4096, 512, 1024, 10
NCORES = 8
BS = B // NCORES          # 512 batch per core
NG = H // 128             # 8 h-tile groups
NJ = N // 128             # 4 n-tile blocks
FS = NG * BS              # 4096
CH = 2                    # batch chunks per core
GB = BS // CH             # 256 batch per chunk
CHF = NG * GB             # 2048 free per chunk tile
INV_SQRT2 = 0.7071067811865476
LAM_SELU = 1.0507009873554805
ALPHA_SELU = 1.6732632423543772
# softsign ~ GT*tanh(u) + GC*clip(u,-1,1)  (fit on [-3.2,3.2]; golden.py)
GT, GC = 0.9896, -0.2706


def _softmax(v):
    v = v - v.max()
    e = np.exp(v)
    return e / e.sum()


def _iter_consts(aw, bw):
    out = []
    for i in range(T):
        w = aw[i]
        c_r = w[1] + w[4] + w[9] + LAM_SELU * w[8] + 0.99 * w[10]
        wE = w[4] + w[9] + LAM_SELU * ALPHA_SELU * w[8]
        c_lin = (w[0] + w[2] + w[5] + 0.01 * w[10] + w[11] + w[12]
                 + 0.5 * w[3] + w[18])
        c_tt = w[15] - w[12] + GT * w[13]
        w6p = w[6] + GC * w[13]
        d = dict(
            c_lin=c_lin, c_r=c_r, wE=wE, ln_wE=float(np.log(wE)),
            c_tt=c_tt, c_ln=w[11] - w[14], w6p=w6p, aw6p=abs(w6p),
            w7=w[7], w17=w[17], w18=w[18], w16=w[16],
            hw3=0.5 * w[3], w19=w[19],
            Ktot=w[16] - wE,
        )
        if i == 0:
            d["coef"], d["szold"] = 1.0, 0.0
        else:
            mom = i / (i + 3.0)
            bwp = bw[i - 1]
            d["coef"] = 1.0 + mom * (1.0 - bwp[1])
            d["szold"] = (-mom * bwp[0]) / d["coef"]
        d["s_xh"] = d["c_lin"] - 1.0 / d["coef"]
        out.append(d)
    return out


def _build(L, aw, bw):
    nc = bacc.Bacc("TRN2", target_bir_lowering=False, debug=False,
                   num_devices=NCORES)
    cons = _iter_consts(aw, bw)

    w_lhs_d = nc.dram_tensor("w_lhs", [128, NJ * H], F16, kind="ExternalInput")
    w_rhs_d = nc.dram_tensor("w_rhs", [128, NJ * H], F16, kind="ExternalInput")
    xT_d = nc.dram_tensor("xT", [128, NJ * BS], F16, kind="ExternalInput")
    identp_d = nc.dram_tensor("identp", [128, 128], F16, kind="ExternalInput")
    identc_d = nc.dram_tensor("identc", [128, (T - 1) * 128], F16,
                              kind="ExternalInput")
    identa_d = nc.dram_tensor("identa", [128, 4 * T * 128], F16,
                              kind="ExternalInput")
    z_d = nc.dram_tensor("z_out", [H, BS], F16, kind="ExternalOutput")

    with tile.TileContext(nc) as tc, ExitStack() as ctx:
        ctx.enter_context(nc.allow_low_precision(
            reason="fp16 basis chain; validated vs jax reference in golden.py"))
        state = ctx.enter_context(tc.tile_pool(name="state", bufs=1))
        psp = ctx.enter_context(tc.tile_pool(name="ps", bufs=1, space="PSUM"))
        psc = [psp.tile([128, CHF], F32, name=f"psc{ch}") for ch in range(CH)]

        m_sb = state.tile([128, NG * H], F16, name="m_sb")
        identp = state.tile([128, 128], F16, name="identp")
        identc = state.tile([128, (T - 1) * 128], F16, name="identc")
        identa = state.tile([128, 4 * T * 128], F16, name="identa")
        c16 = [state.tile([128, CHF], F16, name=f"c16_{ch}") for ch in range(CH)]
        zP = [[state.tile([128, CHF], F16, name=f"z{k}_{ch}") for k in range(2)]
              for ch in range(CH)]
        tmp = [state.tile([128, CHF], F16, name=f"tmp_{ch}") for ch in range(CH)]
        ebias = state.tile([128, T], F32, name="ebias")
        for i in range(T):
            nc.vector.memset(ebias[:, i:i + 1], cons[i]["ln_wE"])
        nc.sync.dma_start(identp[:], identp_d[:, :])
        nc.sync.dma_start(identc[:], identc_d[:, :])
        nc.sync.dma_start(identa[:], identa_d[:, :])

        # ---------------- setup: M = I - W^T W/L (m_sb), c = W^T x/L -------
        with tc.tile_pool(name="setup", bufs=1) as sp:
            w_lhs = sp.tile([128, NJ * H], F16, name="w_lhs")
            w_rhs = sp.tile([128, NJ * H], F16, name="w_rhs")
            xT = sp.tile([128, NJ * BS], F16, name="xT")
            nc.sync.dma_start(w_lhs[:], w_lhs_d[:, :])
            nc.sync.dma_start(w_rhs[:], w_rhs_d[:, :])
            nc.sync.dma_start(xT[:], xT_d[:, :])

            # M in 2 waves of 8 chunks (g,half); wave wv -> m_sb[:, wv*4096:]
            # Uses both psum tiles as one 8-bank scratch.
            for wv in range(2):
                for cg in range(4):
                    g = wv * 4 + cg
                    for half in range(2):
                        bnk = cg * 2 + half
                        pt = psc[bnk // 4]
                        off = (bnk % 4) * 512
                        sl = pt[:, off: off + 512]
                        has_diag = (g * 128) // 512 == half
                        for j in range(NJ):
                            nc.tensor.matmul(
                                sl,
                                w_lhs[:, j * H + g * 128: j * H + g * 128 + 128],
                                w_rhs[:, j * H + half * 512: j * H + half * 512 + 512],
                                start=(j == 0), stop=(j == NJ - 1))
                            if j == 0 and has_diag:
                                doff = off + g * 128 - half * 512
                                nc.tensor.matmul(
                                    pt[:, doff: doff + 128], identp[:],
                                    identp[:], start=False, stop=False,
                                    skip_group_check=True)
                nc.scalar.copy(m_sb[:, wv * 4096: wv * 4096 + 2048], psc[0][:])
                nc.scalar.copy(m_sb[:, wv * 4096 + 2048: (wv + 1) * 4096],
                               psc[1][:])

            # c chunked: psc[ch] group g at [g*GB, (g+1)*GB)
            for ch in range(CH):
                for g in range(NG):
                    sl = psc[ch][:, g * GB:(g + 1) * GB]
                    for j in range(NJ):
                        nc.tensor.matmul(
                            sl,
                            w_lhs[:, j * H + g * 128: j * H + g * 128 + 128],
                            xT[:, j * BS + ch * GB: j * BS + ch * GB + GB],
                            start=(j == 0 and g % 2 == 0), stop=False,
                            skip_group_check=True)
                nc.scalar.copy(c16[ch][:], psc[ch][:])

        # ---------------- iterations ----------------
        work = ctx.enter_context(tc.tile_pool(name="work", bufs=1))

        def wt(tag, ch):
            return work.tile([128, CHF], F16, tag=f"{tag}_{ch}",
                             name=f"{tag}_{ch}")

        z_im1 = [None, None]
        z_im2 = [None, None]
        for i in range(T):
            cc = cons[i]
            coef = cc["coef"]
            tiles = [dict() for _ in range(CH)]

            # --- tmp (GPSIMD) + zg matmuls (PE), per chunk ---
            tsrc = [None, None]
            for ch in range(CH):
                if i == 0:
                    continue
                if i == 1:
                    tsrc[ch] = z_im1[ch]
                else:
                    nc.vector.scalar_tensor_tensor(
                        tmp[ch][:], z_im2[ch][:], cc["szold"], z_im1[ch][:],
                        ALU.mult, ALU.add)
                    tsrc[ch] = tmp[ch]
            for ch in range(CH):
                if i == 0:
                    continue
                # start=True only on the FIRST matmul touching each psum
                # bank (even g): start clears has_written bank-wide, and the
                # odd sibling's fresh region then gets correct
                # overwrite-then-accumulate semantics with start=False.
                for g in range(NG):
                    sl = psc[ch][:, g * GB:(g + 1) * GB]
                    for j in range(NG):
                        nc.tensor.matmul(
                            sl,
                            m_sb[:, j * H + g * 128: j * H + g * 128 + 128],
                            tsrc[ch][:, j * GB:(j + 1) * GB],
                            start=(j == 0 and g % 2 == 0), stop=False,
                            skip_group_check=True)
                    nc.tensor.matmul(
                        sl, identc[:, (i - 1) * 128: i * 128],
                        c16[ch][:, g * GB:(g + 1) * GB],
                        start=False, stop=False, skip_group_check=True)

            # --- DVE: xh + prescales (TS 4x), chunk-ordered ---
            for ch in range(CH):
                t = tiles[ch]
                t["xh"] = wt("xh", ch)
                nc.vector.tensor_scalar(t["xh"][:], psc[ch][:], coef, None,
                                        ALU.mult)
                t["relw"] = wt("relw", ch)
                nc.vector.tensor_scalar(t["relw"][:], t["xh"][:],
                                        cc["c_r"], 0.0, ALU.mult, ALU.max)
                t["c1a"] = wt("c1a", ch)
                nc.vector.tensor_scalar(t["c1a"][:], t["xh"][:], cc["w6p"],
                                        cc["aw6p"], ALU.mult, ALU.min)
                t["c1w"] = wt("c1w", ch)
                nc.vector.tensor_scalar(t["c1w"][:], t["c1a"][:],
                                        -cc["aw6p"], None, ALU.max)
                t["hm1"] = wt("uhw", ch)   # shares buffer w/ uhw (disjoint)
                nc.vector.tensor_scalar(t["hm1"][:], t["xh"][:],
                                        cc["w7"] / 6.0, cc["w7"] / 2.0,
                                        ALU.mult, ALU.add)
                t["hmw"] = wt("hmw", ch)
                nc.vector.tensor_scalar(t["hmw"][:], t["hm1"][:], cc["w7"],
                                        0.0, ALU.min, ALU.max)
                t["thsw"] = wt("thsw", ch)
                nc.vector.tensor_scalar(t["thsw"][:], t["xh"][:],
                                        cc["w17"] / cc["w7"], None, ALU.add)
                t["tsmt"] = wt("tsmt", ch)
                nc.vector.tensor_scalar(t["tsmt"][:], t["xh"][:], -cc["w18"],
                                        -cc["w16"], ALU.mult, ALU.add)
                t["xh35"] = wt("xh35", ch)
                nc.vector.tensor_scalar(t["xh35"][:], t["xh"][:], cc["hw3"],
                                        None, ALU.mult)

            # --- per chunk: ACT [S] -> [L] -> [S] phases, with DVE/GP
            # products interleaved.  Emission order keeps each chunk's set
            # phases contiguous; across chunks the trailing [S] (th) merges
            # with the next chunk's leading [S] block.
            for ch in range(CH):
                t = tiles[ch]
                # order: [2: sm, e2][6: eu, ln][2: tt, th] -- every
                # consumer is >=2 queue slots after its producer, so
                # semaphore-post latency never lets the scheduler shear an
                # op from another set into the middle of a phase.
                t["sm"] = wt("sm", ch)
                nc.scalar.activation(t["sm"][:], psc[ch][:], ACT.Sigmoid,
                                     scale=-coef)
                t["e2"] = wt("e2", ch)
                nc.scalar.activation(t["e2"][:], psc[ch][:], ACT.Erf,
                                     scale=coef * INV_SQRT2)
                t["eu"] = wt("eu", ch)
                nc.scalar.activation(t["eu"][:], psc[ch][:], ACT.Exp,
                                     scale=coef, bias=ebias[:, i:i + 1])
                t["lnsm"] = wt("lnsm", ch)
                nc.scalar.activation(t["lnsm"][:], t["sm"][:], ACT.Ln)
                t["tt"] = wt("tt", ch)
                nc.scalar.activation(t["tt"][:], psc[ch][:], ACT.Tanh,
                                     scale=coef)
                t["th"] = wt("th", ch)
                nc.scalar.activation(t["th"][:], t["lnsm"][:], ACT.Tanh,
                                     scale=-1.0)
                # DVE products + helpers for this chunk
                t["uhw"] = wt("uhw", ch)          # hm1 dead
                nc.vector.tensor_mul(t["uhw"][:], t["thsw"][:], t["hmw"][:])
                t["eum"] = wt("hmw", ch)          # hmw dead after uhw
                nc.vector.tensor_scalar(t["eum"][:], t["eu"][:], cc["wE"],
                                        None, ALU.min)
                t["uth"] = wt("uth", ch)
                nc.vector.tensor_mul(t["uth"][:], t["xh"][:], t["th"][:])
                t["ecr"] = wt("c1a", ch)          # c1a dead after c1w
                nc.vector.tensor_add(t["ecr"][:], t["c1w"][:], t["relw"][:])
                # GPSIMD products
                t["tsm"] = wt("tsm", ch)
                nc.gpsimd.tensor_mul(t["tsm"][:], t["tsmt"][:], t["sm"][:])
                t["ue2w"] = wt("ue2w", ch)
                nc.gpsimd.tensor_mul(t["ue2w"][:], t["xh35"][:], t["e2"][:])

            # --- PE z_op terms continue the open zg psum group (start=False
            # everywhere -> no bank-wide has_written clear -> k-outer order is
            # safe and reuses each stationary across all 8 groups).  The xh
            # stationary is (c_lin - 1/coef) so psum's zg content folds into
            # the linear term: psum + (c_lin-1/coef)*coef*psum = c_lin*u.
            ia = identa[:, (i * 4 + 0) * 128:(i * 4 + 1) * 128]   # c_lin-1/coef
            ib = identa[:, (i * 4 + 1) * 128:(i * 4 + 2) * 128]   # c_tt
            ic = identa[:, (i * 4 + 2) * 128:(i * 4 + 3) * 128]   # c_ln
            id_ = identa[:, (i * 4 + 3) * 128:(i * 4 + 4) * 128]  # w19
            for ch in range(CH):
                t = tiles[ch]
                terms = [(ia, t["xh"]), (ib, t["tt"]), (ic, t["lnsm"]),
                         (identp, t["ecr"]), (identp, t["eum"]),
                         (identp, t["uhw"]), (identp, t["tsm"]),
                         (identp, t["ue2w"]), (id_, t["uth"])]
                for k, (ident_t, src_t) in enumerate(terms):
                    for g in range(NG):
                        nc.tensor.matmul(
                            psc[ch][:, g * GB:(g + 1) * GB],
                            ident_t[:], src_t[:, g * GB:(g + 1) * GB],
                            start=False, stop=(k == len(terms) - 1),
                            skip_group_check=True)

            # --- evac (ACT Copy; copy is in every table set) ---
            for ch in range(CH):
                z_out_t = zP[ch][i % 2]
                nc.scalar.activation(z_out_t[:], psc[ch][:], ACT.Copy,
                                     bias=cc["Ktot"])
                z_im2[ch] = z_im1[ch]
                z_im1[ch] = z_out_t

        for ch in range(CH):
            for g in range(NG):
                nc.sync.dma_start(
                    z_d[g * 128:(g + 1) * 128, ch * GB:(ch + 1) * GB],
                    z_im1[ch][:, g * GB:(g + 1) * GB])

    # Restrict the ACT table-set chooser to sigmoid_and_others (sigmoid,
    # erf, tanh, copy) and natural_log_exp_and_others (ln, exp): the default
    # first-containing-set policy maps tanh/ln to other sets, causing ~8
    # table loads per iteration instead of 2.  Indices are preserved (other
    # entries become empty), so walrus sees valid set ids.  The original
    # function is restored immediately after finalize.
    _orig_gat = bacc.get_activation_tables
    _keep = {"sigmoid_and_others", "natural_log_exp_and_others"}

    def _patched_gat(arch):
        return {name: (fns if name in _keep else set())
                for name, fns in _orig_gat(arch).items()}

    bacc.get_activation_tables = _patched_gat
    try:
        nc.finalize()
    finally:
        bacc.get_activation_tables = _orig_gat
    return nc


_CACHE = {}


def kernel(x, frozen_weight, alpha, layer_beta, _want_trace=False):
    x = np.asarray(x, np.float32)
    frozen_weight = np.asarray(frozen_weight, np.float32)
    alpha = np.asarray(alpha, np.float32)
    layer_beta = np.asarray(layer_beta, np.float32)

    W = frozen_weight[0]                                   # [N, H]
    L = float(np.linalg.norm(W.astype(np.float64), 2) ** 2)
    aw = np.stack([_softmax(alpha[i].astype(np.float64)) for i in range(T)])
    bw = np.stack([_softmax(layer_beta[i].astype(np.float64))
                   for i in range(T)])

    key = (round(L, 10), aw.tobytes(), bw.tobytes())
    if key not in _CACHE:
        _CACHE[key] = _build(L, aw, bw)
    nc = _CACHE[key]
    cons = _iter_consts(aw, bw)

    ident = np.eye(128, dtype=np.float16)
    identc = np.concatenate(
        [ident * np.float16(1.0 / cons[i]["coef"]) for i in range(1, T)],
        axis=1)
    identa = np.concatenate(
        [ident * np.float16(cons[i][k])
         for i in range(T) for k in ("s_xh", "c_tt", "c_ln", "w19")],
        axis=1)

    # w in n-block-partition layout: w_blk[p, j*H + h] = W[j*128+p, h]
    w_blk = W.reshape(NJ, 128, H).transpose(1, 0, 2).reshape(128, NJ * H)
    w_lhs = w_blk.astype(np.float16)
    w_rhs = (-w_blk / L).astype(np.float16)
    xs = x[:, :, 0]                                        # [B, N]

    in_maps = []
    for c in range(NCORES):
        xc = xs[c * BS:(c + 1) * BS, :]                    # [BS, N]
        xT = (xc.T / L).reshape(NJ, 128, BS).transpose(1, 0, 2) \
            .reshape(128, NJ * BS).astype(np.float16)
        in_maps.append({
            "w_lhs": np.ascontiguousarray(w_lhs),
            "w_rhs": np.ascontiguousarray(w_rhs),
            "xT": np.ascontiguousarray(xT),
            "identp": ident,
            "identc": np.ascontiguousarray(identc),
            "identa": np.ascontiguousarray(identa),
        })

    res = run_bass_kernel_spmd(nc, in_maps, list(range(NCORES)),
                               trace=_want_trace)
    # z_out [H, BS] but batch columns are chunk-grouped: column layout is
    # [ch*GB + b] = batch (ch*GB + b)  -- contiguous, nothing to undo.
    z = np.concatenate([np.asarray(res.results[c]["z_out"], np.float32)
                        for c in range(NCORES)], axis=1)
    out = np.ascontiguousarray(z.T)[:, :, None].astype(np.float32)
    if _want_trace:
        return out, res
    return out


if __name__ == "__main__":
    d = np.load('/tmp/inputs.npz')
    out = kernel(d['x'], d['frozen_weight'], d['alpha'], d['layer_beta'])
    ref = np.load('/tmp/ref_out.npy')
    rel = np.linalg.norm(out - ref) / np.linalg.norm(ref)
    print("rel err vs ref:", rel, "absmax:", np.abs(out - ref).max())


# revision 27
# speedup vs baseline: 1.0734x; 1.0180x over previous
"""Trainium2 Bass kernel for nn_DARTSModelLayers (FISTA-style unrolled model).

Math (per reference):
  W = frozen_weight[0]  [N=512, H=1024];  L = ||W||_2^2
  10 iterations of:
    z_aux = z + (i/(i+3)) (z - z_prev)
    z_g   = (I - W^T W / L) z_aux + W^T x / L
    z_op  = sum_k softmax(alpha_i)_k * op_k(z_g)        (20 activations)
    z_prev = bw0 z + bw1 z_op ; z = z_op

Per-iteration decomposition (u = z_g; psum = A@tmp + c/coef; u = coef*psum):
  ACT (6 passes, 2 table loads): sm=sigmoid(-u), e2=erf(u/sqrt2), tt=tanh(u)
    [sigmoid set]; lnsm=ln(sm), eu=exp(u+ln wE) [nat_log_exp set];
    th=tanh(-lnsm) [sigmoid set again -> next iter needs no load]
  DVE (TS 4x / TT 2x fp16): xh=coef*psum, tmp (STT), prescales (c_r*relu,
    clip, hardsig, (u+w17/w7), (-w18*u-w16), hw3*u), eum=min(eu,wE) (the
    exp(min(u,0))=min(exp(u),1) trick), products uhw/uth, ecr=c1w+relw.
  GPSIMD (TENSOR_TENSOR only on TRN2): products tsm, ue2w.
  PE: zg matmuls (A fp16 with I folded; c via scaled-identity matmul), then
    z_op terms continue the same psum accumulation group via scaled-identity
    matmuls (start=True only on each bank's first touch; the xh stationary
    is c_lin-1/coef so the zg psum content folds into the linear term).
  ACT order [sm,e2][eu,ln][tt,th] keeps every consumer >=2 slots behind its
  producer so the scheduler cannot shear table-set phases (4 loads/iter);
  the set chooser is restricted to sigmoid_and_others/natural_log_exp via a
  scoped get_activation_tables patch around finalize.  Evac is an ACT Copy
  with bias=Ktot.

Softsign is approximated as GT*tanh(u) + GC*clip(u,-1,1), folded into the
tanh / hardtanh coefficients (golden.py: end-to-end rel err 4.3e-3 fp16).

Batch columns are independent through the whole recurrence, so each core
processes its 512 batch columns as CH=2 chunks of 256, each chunk owning
half of PSUM (4 banks). Chunk B's tail overlaps chunk A's next-iteration
matmuls, keeping all four engines busy across the serial iteration chain.

Sharding: batch B=4096 split over 8 cores (512 each); weights replicated.
Output is produced in [H, B_shard] fp16 layout; host transposes to [B,H,1].
"""
import sys
import numpy as np

sys.path.insert(0, "/opt/trn_rl_repo")

import concourse.bass as bass  # noqa: E402
import concourse.bacc as bacc  # noqa: E402
import concourse.tile as tile  # noqa: E402
from concourse import mybir  # noqa: E402
from concourse.bass_utils import run_bass_kernel_spmd  # noqa: E402
from contextlib import ExitStack  # noqa: E402

F32 = mybir.dt.float32
F16 = mybir.dt.float16
ACT = mybir.ActivationFunctionType
ALU = mybir.AluOpType

B, N, H, T = 4096, 512, 1024, 10
NCORES = 8
BS = B // NCORES          # 512 batch per core
NG = H // 128             # 8 h-tile groups
NJ = N // 128             # 4 n-tile blocks
FS = NG * BS              # 4096
CH = 2                    # batch chunks per core
GB = BS // CH             # 256 batch per chunk
CHF = NG * GB             # 2048 free per chunk tile
INV_SQRT2 = 0.7071067811865476
LAM_SELU = 1.0507009873554805
ALPHA_SELU = 1.6732632423543772
# softsign ~ GT*tanh(u) + GC*clip(u,-1,1)  (fit on [-3.2,3.2]; golden.py)
GT, GC = 0.9896, -0.2706


def _softmax(v):
    v = v - v.max()
    e = np.exp(v)
    return e / e.sum()


def _iter_consts(aw, bw):
    out = []
    for i in range(T):
        w = aw[i]
        c_r = w[1] + w[4] + w[9] + LAM_SELU * w[8] + 0.99 * w[10]
        wE = w[4] + w[9] + LAM_SELU * ALPHA_SELU * w[8]
        c_lin = (w[0] + w[2] + w[5] + 0.01 * w[10] + w[11] + w[12]
                 + 0.5 * w[3] + w[18])
        c_tt = w[15] - w[12] + GT * w[13]
        w6p = w[6] + GC * w[13]
        d = dict(
            c_lin=c_lin, c_r=c_r, wE=wE, ln_wE=float(np.log(wE)),
            c_tt=c_tt, c_ln=w[11] - w[14], w6p=w6p, aw6p=abs(w6p),
            w7=w[7], w17=w[17], w18=w[18], w16=w[16],
            hw3=0.5 * w[3], w19=w[19],
            Ktot=w[16] - wE,
        )
        if i == 0:
            d["coef"], d["szold"] = 1.0, 0.0
        else:
            mom = i / (i + 3.0)
            bwp = bw[i - 1]
            d["coef"] = 1.0 + mom * (1.0 - bwp[1])
            d["szold"] = (-mom * bwp[0]) / d["coef"]
        d["s_xh"] = d["c_lin"] - 1.0 / d["coef"]
        out.append(d)
    return out


def _build(L, aw, bw):
    nc = bacc.Bacc("TRN2", target_bir_lowering=False, debug=False,
                   num_devices=NCORES)
    cons = _iter_consts(aw, bw)

    w_lhs_d = nc.dram_tensor("w_lhs", [128, NJ * H], F16, kind="ExternalInput")
    w_rhs_d = nc.dram_tensor("w_rhs", [128, NJ * H], F16, kind="ExternalInput")
    xT_d = nc.dram_tensor("xT", [128, NJ * BS], F16, kind="ExternalInput")
    identp_d = nc.dram_tensor("identp", [128, 128], F16, kind="ExternalInput")
    identc_d = nc.dram_tensor("identc", [128, (T - 1) * 128], F16,
                              kind="ExternalInput")
    identa_d = nc.dram_tensor("identa", [128, 4 * T * 128], F16,
                              kind="ExternalInput")
    z_d = nc.dram_tensor("z_out", [H, BS], F16, kind="ExternalOutput")

    with tile.TileContext(nc) as tc, ExitStack() as ctx:
        ctx.enter_context(nc.allow_low_precision(
            reason="fp16 basis chain; validated vs jax reference in golden.py"))
        state = ctx.enter_context(tc.tile_pool(name="state", bufs=1))
        psp = ctx.enter_context(tc.tile_pool(name="ps", bufs=1, space="PSUM"))
        psc = [psp.tile([128, CHF], F32, name=f"psc{ch}") for ch in range(CH)]

        m_sb = state.tile([128, NG * H], F16, name="m_sb")
        identp = state.tile([128, 128], F16, name="identp")
        identc = state.tile([128, (T - 1) * 128], F16, name="identc")
        identa = state.tile([128, 4 * T * 128], F16, name="identa")
        c16 = [state.tile([128, CHF], F16, name=f"c16_{ch}") for ch in range(CH)]
        zP = [[state.tile([128, CHF], F16, name=f"z{k}_{ch}") for k in range(2)]
              for ch in range(CH)]
        tmp = [state.tile([128, CHF], F16, name=f"tmp_{ch}") for ch in range(CH)]
        ebias = state.tile([128, T], F32, name="ebias")
        for i in range(T):
            nc.vector.memset(ebias[:, i:i + 1], cons[i]["ln_wE"])
        nc.sync.dma_start(identp[:], identp_d[:, :])
        nc.sync.dma_start(identc[:], identc_d[:, :])
        nc.sync.dma_start(identa[:], identa_d[:, :])

        # ---------------- setup: M = I - W^T W/L (m_sb), c = W^T x/L -------
        with tc.tile_pool(name="setup", bufs=1) as sp:
            w_lhs = sp.tile([128, NJ * H], F16, name="w_lhs")
            w_rhs = sp.tile([128, NJ * H], F16, name="w_rhs")
            xT = sp.tile([128, NJ * BS], F16, name="xT")
            nc.sync.dma_start(w_lhs[:], w_lhs_d[:, :])
            nc.sync.dma_start(w_rhs[:], w_rhs_d[:, :])
            nc.sync.dma_start(xT[:], xT_d[:, :])

            # M in 2 waves of 8 chunks (g,half); wave wv -> m_sb[:, wv*4096:]
            # Uses both psum tiles as one 8-bank scratch.
            for wv in range(2):
                for cg in range(4):
                    g = wv * 4 + cg
                    for half in range(2):
                        bnk = cg * 2 + half
                        pt = psc[bnk // 4]
                        off = (bnk % 4) * 512
                        sl = pt[:, off: off + 512]
                        has_diag = (g * 128) // 512 == half
                        for j in range(NJ):
                            nc.tensor.matmul(
                                sl,
                                w_lhs[:, j * H + g * 128: j * H + g * 128 + 128],
                                w_rhs[:, j * H + half * 512: j * H + half * 512 + 512],
                                start=(j == 0), stop=(j == NJ - 1))
                            if j == 0 and has_diag:
                                doff = off + g * 128 - half * 512
                                nc.tensor.matmul(
                                    pt[:, doff: doff + 128], identp[:],
                                    identp[:], start=False, stop=False,
                                    skip_group_check=True)
                nc.scalar.copy(m_sb[:, wv * 4096: wv * 4096 + 2048], psc[0][:])
                nc.scalar.copy(m_sb[:, wv * 4096 + 2048: (wv + 1) * 4096],
                               psc[1][:])

            # c chunked: psc[ch] group g at [g*GB, (g+1)*GB)
            for ch in range(CH):
                for g in range(NG):
                    sl = psc[ch][:, g * GB:(g + 1) * GB]
                    for j in range(NJ):
                        nc.tensor.matmul(
                            sl,
                            w_lhs[:, j * H + g * 128: j * H + g * 128 + 128],
                            xT[:, j * BS + ch * GB: j * BS + ch * GB + GB],
                            start=(j == 0 and g % 2 == 0), stop=False,
                            skip_group_check=True)
                nc.scalar.copy(c16[ch][:], psc[ch][:])

        # ---------------- iterations ----------------
        work = ctx.enter_context(tc.tile_pool(name="work", bufs=1))

        def wt(tag, ch):
            return work.tile([128, CHF], F16, tag=f"{tag}_{ch}",
                             name=f"{tag}_{ch}")

        z_im1 = [None, None]
        z_im2 = [None, None]
        for i in range(T):
            cc = cons[i]
            coef = cc["coef"]
            tiles = [dict() for _ in range(CH)]

            # --- zg matmuls (PE); tmp[ch] was produced at the end of the
            # previous iteration directly from psum, so zg does not wait on
            # the previous evac.
            for ch in range(CH):
                if i == 0:
                    continue
                # start=True only on the FIRST matmul touching each psum
                # bank (even g): start clears has_written bank-wide, and the
                # odd sibling's fresh region then gets correct
                # overwrite-then-accumulate semantics with start=False.
                for g in range(NG):
                    sl = psc[ch][:, g * GB:(g + 1) * GB]
                    for j in range(NG):
                        nc.tensor.matmul(
                            sl,
                            m_sb[:, j * H + g * 128: j * H + g * 128 + 128],
                            tmp[ch][:, j * GB:(j + 1) * GB],
                            start=(j == 0 and g % 2 == 0), stop=False,
                            skip_group_check=True)
                    nc.tensor.matmul(
                        sl, identc[:, (i - 1) * 128: i * 128],
                        c16[ch][:, g * GB:(g + 1) * GB],
                        start=False, stop=False, skip_group_check=True)

            # --- DVE: xh + prescales (TS 4x), chunk-ordered ---
            for ch in range(CH):
                t = tiles[ch]
                t["xh"] = wt("xh", ch)
                nc.vector.tensor_scalar(t["xh"][:], psc[ch][:], coef, None,
                                        ALU.mult)
                t["relw"] = wt("relw", ch)
                nc.vector.tensor_scalar(t["relw"][:], t["xh"][:],
                                        cc["c_r"], 0.0, ALU.mult, ALU.max)
                t["c1a"] = wt("c1a", ch)
                nc.vector.tensor_scalar(t["c1a"][:], t["xh"][:], cc["w6p"],
                                        cc["aw6p"], ALU.mult, ALU.min)
                t["c1w"] = wt("c1w", ch)
                nc.vector.tensor_scalar(t["c1w"][:], t["c1a"][:],
                                        -cc["aw6p"], None, ALU.max)
                t["hm1"] = wt("uhw", ch)   # shares buffer w/ uhw (disjoint)
                nc.vector.tensor_scalar(t["hm1"][:], t["xh"][:],
                                        cc["w7"] / 6.0, cc["w7"] / 2.0,
                                        ALU.mult, ALU.add)
                t["hmw"] = wt("hmw", ch)
                nc.vector.tensor_scalar(t["hmw"][:], t["hm1"][:], cc["w7"],
                                        0.0, ALU.min, ALU.max)
                t["thsw"] = wt("thsw", ch)
                nc.vector.tensor_scalar(t["thsw"][:], t["xh"][:],
                                        cc["w17"] / cc["w7"], None, ALU.add)
                t["tsmt"] = wt("tsmt", ch)
                nc.vector.tensor_scalar(t["tsmt"][:], t["xh"][:], -cc["w18"],
                                        -cc["w16"], ALU.mult, ALU.add)
                t["xh35"] = wt("xh35", ch)
                nc.vector.tensor_scalar(t["xh35"][:], t["xh"][:], cc["hw3"],
                                        None, ALU.mult)

            # --- per chunk: ACT [S] -> [L] -> [S] phases, with DVE/GP
            # products interleaved.  Emission order keeps each chunk's set
            # phases contiguous; across chunks the trailing [S] (th) merges
            # with the next chunk's leading [S] block.
            for ch in range(CH):
                t = tiles[ch]
                # order: [2: sm, e2][6: eu, ln][2: tt, th] -- every
                # consumer is >=2 queue slots after its producer, so
                # semaphore-post latency never lets the scheduler shear an
                # op from another set into the middle of a phase.
                t["sm"] = wt("sm", ch)
                nc.scalar.activation(t["sm"][:], psc[ch][:], ACT.Sigmoid,
                                     scale=-coef)
                t["e2"] = wt("e2", ch)
                nc.scalar.activation(t["e2"][:], psc[ch][:], ACT.Erf,
                                     scale=coef * INV_SQRT2)
                t["eu"] = wt("eu", ch)
                nc.scalar.activation(t["eu"][:], psc[ch][:], ACT.Exp,
                                     scale=coef, bias=ebias[:, i:i + 1])
                t["lnsm"] = wt("lnsm", ch)
                nc.scalar.activation(t["lnsm"][:], t["sm"][:], ACT.Ln)
                t["tt"] = wt("tt", ch)
                nc.scalar.activation(t["tt"][:], psc[ch][:], ACT.Tanh,
                                     scale=coef)
                t["th"] = wt("th", ch)
                nc.scalar.activation(t["th"][:], t["lnsm"][:], ACT.Tanh,
                                     scale=-1.0)
                # DVE products + helpers for this chunk
                t["uhw"] = wt("uhw", ch)          # hm1 dead
                nc.vector.tensor_mul(t["uhw"][:], t["thsw"][:], t["hmw"][:])
                t["eum"] = wt("hmw", ch)          # hmw dead after uhw
                nc.vector.tensor_scalar(t["eum"][:], t["eu"][:], cc["wE"],
                                        None, ALU.min)
                t["uth"] = wt("uth", ch)
                nc.vector.tensor_mul(t["uth"][:], t["xh"][:], t["th"][:])
                t["ecr"] = wt("c1a", ch)          # c1a dead after c1w
                nc.vector.tensor_add(t["ecr"][:], t["c1w"][:], t["relw"][:])
                if 0 < i < T - 1:
                    t["sz2"] = wt("th", ch)       # th dead after uth
                    nc.vector.tensor_scalar(
                        t["sz2"][:], z_im1[ch][:],
                        cons[i + 1]["szold"], None, ALU.mult)
                # GPSIMD products
                t["tsm"] = wt("tsm", ch)
                nc.gpsimd.tensor_mul(t["tsm"][:], t["tsmt"][:], t["sm"][:])
                t["ue2w"] = wt("ue2w", ch)
                nc.gpsimd.tensor_mul(t["ue2w"][:], t["xh35"][:], t["e2"][:])

            # --- PE z_op terms continue the open zg psum group (start=False
            # everywhere -> no bank-wide has_written clear -> k-outer order is
            # safe and reuses each stationary across all 8 groups).  The xh
            # stationary is (c_lin - 1/coef) so psum's zg content folds into
            # the linear term: psum + (c_lin-1/coef)*coef*psum = c_lin*u.
            ia = identa[:, (i * 4 + 0) * 128:(i * 4 + 1) * 128]   # c_lin-1/coef
            ib = identa[:, (i * 4 + 1) * 128:(i * 4 + 2) * 128]   # c_tt
            ic = identa[:, (i * 4 + 2) * 128:(i * 4 + 3) * 128]   # c_ln
            id_ = identa[:, (i * 4 + 3) * 128:(i * 4 + 4) * 128]  # w19
            for ch in range(CH):
                t = tiles[ch]
                terms = [(ia, t["xh"]), (identp, t["ecr"]),
                         (identp, t["uhw"]), (identp, t["eum"]),
                         (ic, t["lnsm"]), (ib, t["tt"]),
                         (identp, t["tsm"]), (identp, t["ue2w"]),
                         (id_, t["uth"])]
                for k, (ident_t, src_t) in enumerate(terms):
                    for g in range(NG):
                        nc.tensor.matmul(
                            psc[ch][:, g * GB:(g + 1) * GB],
                            ident_t[:], src_t[:, g * GB:(g + 1) * GB],
                            start=False, stop=(k == len(terms) - 1),
                            skip_group_check=True)

            # --- next-iter tmp straight from psum (parallel with evac) ---
            for ch in range(CH):
                if i == T - 1:
                    continue
                if i == 0:
                    nc.vector.tensor_scalar(tmp[ch][:], psc[ch][:], 1.0,
                                            cc["Ktot"], ALU.mult, ALU.add)
                else:
                    nc.vector.scalar_tensor_tensor(
                        tmp[ch][:], psc[ch][:], cc["Ktot"],
                        tiles[ch]["sz2"][:], ALU.add, ALU.add)

            # --- evac (ACT Copy; copy is in every table set) ---
            for ch in range(CH):
                z_out_t = zP[ch][i % 2]
                nc.scalar.activation(z_out_t[:], psc[ch][:], ACT.Copy,
                                     bias=cc["Ktot"])
                z_im2[ch] = z_im1[ch]
                z_im1[ch] = z_out_t

        for ch in range(CH):
            for g in range(NG):
                nc.sync.dma_start(
                    z_d[g * 128:(g + 1) * 128, ch * GB:(ch + 1) * GB],
                    z_im1[ch][:, g * GB:(g + 1) * GB])

    # Restrict the ACT table-set chooser to sigmoid_and_others (sigmoid,
    # erf, tanh, copy) and natural_log_exp_and_others (ln, exp): the default
    # first-containing-set policy maps tanh/ln to other sets, causing ~8
    # table loads per iteration instead of 2.  Indices are preserved (other
    # entries become empty), so walrus sees valid set ids.  The original
    # function is restored immediately after finalize.
    _orig_gat = bacc.get_activation_tables
    _keep = {"sigmoid_and_others", "natural_log_exp_and_others"}

    def _patched_gat(arch):
        return {name: (fns if name in _keep else set())
                for name, fns in _orig_gat(arch).items()}

    bacc.get_activation_tables = _patched_gat
    try:
        nc.finalize()
    finally:
        bacc.get_activation_tables = _orig_gat
    return nc


_CACHE = {}


def kernel(x, frozen_weight, alpha, layer_beta, _want_trace=False):
    x = np.asarray(x, np.float32)
    frozen_weight = np.asarray(frozen_weight, np.float32)
    alpha = np.asarray(alpha, np.float32)
    layer_beta = np.asarray(layer_beta, np.float32)

    W = frozen_weight[0]                                   # [N, H]
    L = float(np.linalg.norm(W.astype(np.float64), 2) ** 2)
    aw = np.stack([_softmax(alpha[i].astype(np.float64)) for i in range(T)])
    bw = np.stack([_softmax(layer_beta[i].astype(np.float64))
                   for i in range(T)])

    key = (round(L, 10), aw.tobytes(), bw.tobytes())
    if key not in _CACHE:
        _CACHE[key] = _build(L, aw, bw)
    nc = _CACHE[key]
    cons = _iter_consts(aw, bw)

    ident = np.eye(128, dtype=np.float16)
    identc = np.concatenate(
        [ident * np.float16(1.0 / cons[i]["coef"]) for i in range(1, T)],
        axis=1)
    identa = np.concatenate(
        [ident * np.float16(cons[i][k])
         for i in range(T) for k in ("s_xh", "c_tt", "c_ln", "w19")],
        axis=1)

    # w in n-block-partition layout: w_blk[p, j*H + h] = W[j*128+p, h]
    w_blk = W.reshape(NJ, 128, H).transpose(1, 0, 2).reshape(128, NJ * H)
    w_lhs = w_blk.astype(np.float16)
    w_rhs = (-w_blk / L).astype(np.float16)
    xs = x[:, :, 0]                                        # [B, N]

    in_maps = []
    for c in range(NCORES):
        xc = xs[c * BS:(c + 1) * BS, :]                    # [BS, N]
        xT = (xc.T / L).reshape(NJ, 128, BS).transpose(1, 0, 2) \
            .reshape(128, NJ * BS).astype(np.float16)
        in_maps.append({
            "w_lhs": np.ascontiguousarray(w_lhs),
            "w_rhs": np.ascontiguousarray(w_rhs),
            "xT": np.ascontiguousarray(xT),
            "identp": ident,
            "identc": np.ascontiguousarray(identc),
            "identa": np.ascontiguousarray(identa),
        })

    res = run_bass_kernel_spmd(nc, in_maps, list(range(NCORES)),
                               trace=_want_trace)
    # z_out [H, BS] but batch columns are chunk-grouped: column layout is
    # [ch*GB + b] = batch (ch*GB + b)  -- contiguous, nothing to undo.
    z = np.concatenate([np.asarray(res.results[c]["z_out"], np.float32)
                        for c in range(NCORES)], axis=1)
    out = np.ascontiguousarray(z.T)[:, :, None].astype(np.float32)
    if _want_trace:
        return out, res
    return out


if __name__ == "__main__":
    d = np.load('/tmp/inputs.npz')
    out = kernel(d['x'], d['frozen_weight'], d['alpha'], d['layer_beta'])
    ref = np.load('/tmp/ref_out.npy')
    rel = np.linalg.norm(out - ref) / np.linalg.norm(ref)
    print("rel err vs ref:", rel, "absmax:", np.abs(out - ref).max())
